# revision 2
# baseline (speedup 1.0000x reference)
"""Trainium2 Bass kernel for nn_LogitsProjector.

Computes out[N_TOK, SV] = teacher_logits[N_TOK, K] @ projection[SV, K].T
as a column-parallel (student_vocab-sharded) SPMD GEMM across 8 NeuronCores.

Per core: C[2048, 3200] = A[2048, 32000] @ B_shard[32000, 3200]
  - fp16 inputs (hw-measured rms rel err ~3e-4), fp32 PSUM accumulation.
  - Blocks of Bm=512 x Bn=800: 4 PSUM tiles of [128, 800] (2 banks each)
    accumulate over the full K=32000 (250 k-tiles) before eviction.
  - Both operand panels streamed from HBM in chunks of CK k-tiles;
    traffic = 4 sweeps of each operand = ~1.35 GB/core << compute time.
"""

import numpy as np

P = 128
N_TOK = 2048          # GEMM M
K = 32000             # contraction (teacher vocab)
SV = 25000            # student vocab (total output cols)
N_CORES = 8
NPC = 3200            # output cols per core (25600 padded / 8)
MB = 512              # m block (4 subtiles of 128)
NB = 800              # n block (512 + 288 matmul slices)
M_BLKS = N_TOK // MB  # 4
N_BLKS = NPC // NB    # 4
KO = K // P           # 250 k-tiles
CK = 10               # k-tiles per DMA chunk

_cache = {}


def _build(m_blks=M_BLKS, n_blks=N_BLKS):
    import concourse.bacc as bacc
    import concourse.mybir as mybir
    import concourse.tile as tile

    f16 = mybir.dt.float16
    f32 = mybir.dt.float32

    nc = bacc.Bacc(None, target_bir_lowering=False, debug=False)
    kxm = nc.dram_tensor("kxm", (P, M_BLKS, KO, MB), f16, kind="ExternalInput")
    kxn = nc.dram_tensor("kxn", (P, N_BLKS, KO, NB), f16, kind="ExternalInput")
    out = nc.dram_tensor("out", (P, N_TOK // P, NPC), f32, kind="ExternalOutput")

    with tile.TileContext(nc) as tc:
        with tc.tile_pool(name="apool", bufs=3) as apool, \
             tc.tile_pool(name="bpool", bufs=3) as bpool, \
             tc.tile_pool(name="opool", bufs=4) as opool, \
             tc.tile_pool(name="pspool", bufs=1, space="PSUM") as pspool:
            for mb in range(m_blks):
                for nb in range(n_blks):
                    ps = [pspool.tile([P, NB], f32, name=f"ps{s}") for s in range(4)]
                    for kc in range(KO // CK):
                        at = apool.tile([P, CK, MB], f16, name="a")
                        bt = bpool.tile([P, CK, NB], f16, name="b")
                        nc.sync.dma_start(at[:], kxm[:, mb, kc * CK:(kc + 1) * CK, :])
                        nc.sync.dma_start(bt[:], kxn[:, nb, kc * CK:(kc + 1) * CK, :])
                        for ki in range(CK):
                            kg = kc * CK + ki
                            st, sp = kg == 0, kg == KO - 1
                            for ms in range(4):
                                lhsT = at[:, ki, ms * 128:(ms + 1) * 128]
                                nc.tensor.matmul(ps[ms][:, 0:512], lhsT,
                                                 bt[:, ki, 0:512], start=st, stop=sp)
                                nc.tensor.matmul(ps[ms][:, 512:NB], lhsT,
                                                 bt[:, ki, 512:NB], start=st, stop=sp)
                    for ms in range(4):
                        ot = opool.tile([P, NB], f32, name="o")
                        nc.vector.tensor_copy(ot[:], ps[ms][:])
                        nc.sync.dma_start(
                            out[:, mb * 4 + ms, nb * NB:(nb + 1) * NB], ot[:])
    nc.compile()
    return nc


def _get_nc():
    if "nc" not in _cache:
        _cache["nc"] = _build()
    return _cache["nc"]


def _prep_kxm(teacher: np.ndarray) -> np.ndarray:
    # kxm[p, mb, ko, mc] = teacher[mb*MB + mc, ko*P + p]
    t = teacher.astype(np.float16)
    return np.ascontiguousarray(
        t.reshape(M_BLKS, MB, KO, P).transpose(3, 0, 2, 1))


def _prep_kxn(shard: np.ndarray) -> np.ndarray:
    # kxn[p, nb, ko, nc] = shard[nb*NB + nc, ko*P + p]
    s = shard.astype(np.float16)
    return np.ascontiguousarray(
        s.reshape(N_BLKS, NB, KO, P).transpose(3, 0, 2, 1))


def kernel(teacher_logits: np.ndarray, projection: np.ndarray) -> np.ndarray:
    from concourse.bass_utils import run_bass_kernel_spmd

    nc = _get_nc()

    kxm_np = _prep_kxm(np.asarray(teacher_logits))
    proj = np.asarray(projection).astype(np.float16)
    proj_pad = np.zeros((N_CORES * NPC, K), dtype=np.float16)
    proj_pad[:SV] = proj

    in_maps = []
    for c in range(N_CORES):
        shard = proj_pad[c * NPC:(c + 1) * NPC]
        in_maps.append({"kxm": kxm_np, "kxn": _prep_kxn(shard)})

    res = run_bass_kernel_spmd(nc, in_maps, core_ids=list(range(N_CORES)))
    _cache["last_res"] = res

    parts = []
    for c in range(N_CORES):
        o = res.results[c]["out"]  # (P, N_TOK//P, NPC)
        parts.append(o.transpose(1, 0, 2).reshape(N_TOK, NPC))
    full = np.concatenate(parts, axis=1)[:, :SV]
    return np.ascontiguousarray(full.astype(np.float32))
